# revision 15
# baseline (speedup 1.0000x reference)
"""MoE routing kernel (top-2 of 32 experts, dense-mix form) for 8 TRN2 cores.

Math identity used: out = sum_e mix_w[:, e] * (x @ W_e) + mix_b @ expert_biases,
where mix_w / mix_b are the dense top-2 softmax mixture coefficients from the
two routers. Experts are sharded 4-per-core; each core computes a partial sum
(including 1/8 of the bias term) and the host adds the 8 partials.
"""

import sys

if "/opt/trn_rl_repo" not in sys.path:
    sys.path.insert(0, "/opt/trn_rl_repo")

from contextlib import ExitStack

import ml_dtypes
import numpy as np

import concourse.bacc as bacc
import concourse.tile as tile
from concourse import mybir
from concourse.bass_utils import run_bass_kernel_spmd

B = 128        # batch
D = 1024       # in = out features
E = 32         # experts
NCORES = 8
EPC = E // NCORES   # experts per core
KT = D // 128       # k-tiles of 128 along contraction dim
HD = 512            # psum-bank-sized output chunk

F32 = mybir.dt.float32
BF16 = mybir.dt.bfloat16
ALU = mybir.AluOpType
ACTF = mybir.ActivationFunctionType


def _ctile(pool, name, shape, dtype):
    # unique tag => dedicated slot, never rotated/reused
    return pool.tile(shape, dtype, name=name, tag=name)


def build_program(reps=1):
    nc = bacc.Bacc("TRN2")

    xt32_d = nc.dram_tensor("xt32", [128, KT, B], F32, kind="ExternalInput")
    xtb_d = nc.dram_tensor("xtb", [128, KT, B], BF16, kind="ExternalInput")
    rw2_d = nc.dram_tensor("rw2", [128, KT, 2 * E], F32, kind="ExternalInput")
    wloc_d = nc.dram_tensor("wloc", [EPC, 128, KT, D], BF16, kind="ExternalInput")
    bsc_d = nc.dram_tensor("bsc", [E, D], F32, kind="ExternalInput")
    sel_d = nc.dram_tensor("sel", [E, EPC], F32, kind="ExternalInput")
    id_d = nc.dram_tensor("ident", [128, 128], F32, kind="ExternalInput")
    out_d = nc.dram_tensor("out", [B, D], F32, kind="ExternalOutput")

    with ExitStack() as ctx:
        tc = ctx.enter_context(tile.TileContext(nc))
        const = ctx.enter_context(tc.tile_pool(name="const", bufs=1))
        wpool = ctx.enter_context(tc.tile_pool(name="wts", bufs=EPC))
        pp_a = ctx.enter_context(tc.tile_pool(name="pa", bufs=1, space="PSUM"))
        pp_t = ctx.enter_context(tc.tile_pool(name="pt", bufs=1, space="PSUM"))
        pp_b = ctx.enter_context(tc.tile_pool(name="pb", bufs=1, space="PSUM"))
        pp_e = ctx.enter_context(tc.tile_pool(name="pe", bufs=2, space="PSUM"))

        for _ in range(reps):
            # ---- input DMAs (router-critical consts first, then weights) ----
            xt32 = _ctile(const, "xt32", [128, KT, B], F32)
            nc.sync.dma_start(xt32[:], xt32_d[:])
            rw2 = _ctile(const, "rw2", [128, KT, 2 * E], F32)
            nc.sync.dma_start(rw2[:], rw2_d[:])
            xtb = _ctile(const, "xtb", [128, KT, B], BF16)
            nc.sync.dma_start(xtb[:], xtb_d[:])

            wts = []
            w0 = wpool.tile([128, KT, D], BF16, name="w")
            nc.sync.dma_start(w0[:], wloc_d[0])
            wts.append(w0)

            bsc = _ctile(const, "bsc", [E, D], F32)
            nc.sync.dma_start(bsc[:], bsc_d[:])
            sel = _ctile(const, "sel", [E, EPC], F32)
            nc.sync.dma_start(sel[:], sel_d[:])
            ident = _ctile(const, "ident", [128, 128], F32)
            nc.sync.dma_start(ident[:], id_d[:])

            for e in range(1, EPC):
                w = wpool.tile([128, KT, D], BF16, name="w")
                nc.sync.dma_start(w[:], wloc_d[e])
                wts.append(w)

            # ---- router logits: [B, 64] = x @ [router_w | bias_router_w] ----
            pl = pp_a.tile([B, 2 * E], F32, name="pa")
            for k in range(KT):
                nc.tensor.matmul(
                    pl[:], xt32[:, k, :], rw2[:, k, :],
                    start=(k == 0), stop=(k == KT - 1),
                )
            logits = _ctile(const, "logits", [B, 2 * E], F32)
            nc.scalar.copy(logits[:], pl[:])

            # ---- top-2 + softmax per half -> dense mix coeffs [B, 64] ----
            mix_comb = _ctile(const, "mix_comb", [B, 2 * E], F32)
            for h in range(2):
                lh = logits[:, h * E:(h + 1) * E]
                mx1 = _ctile(const, f"mx1_{h}", [B, 1], F32)
                nc.vector.tensor_reduce(mx1[:], lh, axis=mybir.AxisListType.X, op=ALU.max)
                m1 = _ctile(const, f"m1_{h}", [B, E], F32)
                nc.vector.tensor_scalar(m1[:], lh, mx1[:], None, op0=ALU.is_ge)
                msk = _ctile(const, f"msk_{h}", [B, E], F32)
                nc.vector.scalar_tensor_tensor(
                    msk[:], m1[:], -1e30, lh, op0=ALU.mult, op1=ALU.add
                )
                mx2 = _ctile(const, f"mx2_{h}", [B, 1], F32)
                nc.vector.tensor_reduce(mx2[:], msk[:], axis=mybir.AxisListType.X, op=ALU.max)
                m2 = _ctile(const, f"m2_{h}", [B, E], F32)
                nc.vector.tensor_scalar(m2[:], msk[:], mx2[:], None, op0=ALU.is_ge)
                dgap = _ctile(const, f"dgap_{h}", [B, 1], F32)
                nc.vector.tensor_sub(dgap[:], mx2[:], mx1[:])
                ed = _ctile(const, f"ed_{h}", [B, 1], F32)
                nc.scalar.activation(ed[:], dgap[:], ACTF.Exp)
                den = _ctile(const, f"den_{h}", [B, 1], F32)
                nc.vector.tensor_scalar_add(den[:], ed[:], 1.0)
                p1 = _ctile(const, f"p1_{h}", [B, 1], F32)
                nc.vector.reciprocal(p1[:], den[:])
                p2 = _ctile(const, f"p2_{h}", [B, 1], F32)
                nc.vector.tensor_mul(p2[:], ed[:], p1[:])
                t2 = _ctile(const, f"t2_{h}", [B, E], F32)
                nc.vector.tensor_scalar_mul(t2[:], m2[:], p2[:])
                nc.vector.scalar_tensor_tensor(
                    mix_comb[:, h * E:(h + 1) * E], m1[:], p1[:], t2[:],
                    op0=ALU.mult, op1=ALU.add,
                )

            # ---- transpose each mix half to [32, B] for use as matmul lhsT ----
            # (two transposes so both land at base partition 0, required by PE)
            pt0 = pp_t.tile([E, B], F32, name="pt")
            nc.tensor.transpose(pt0[:], mix_comb[:, 0:E], ident[:])
            mixTw = _ctile(const, "mixTw", [E, B], F32)
            nc.scalar.copy(mixTw[:], pt0[:])
            pt1 = pp_t.tile([E, B], F32, name="pt")
            nc.tensor.transpose(pt1[:], mix_comb[:, E:2 * E], ident[:])
            mixTb = _ctile(const, "mixTb", [E, B], F32)
            nc.scalar.copy(mixTb[:], pt1[:])

            # ---- this core's 4 mix coefficients: mixT_w.T @ sel -> [B, 4] ----
            pml = pp_a.tile([B, 2 * E], F32, name="pa")
            nc.tensor.matmul(pml[:, 0:EPC], mixTw[:], sel[:], start=True, stop=True)
            mix_loc = _ctile(const, "mix_loc", [B, EPC], F32)
            nc.scalar.copy(mix_loc[:], pml[:, 0:EPC])

            # ---- bias term (pre-scaled by 1/8 on host): mix_b @ bsc ----
            pb = pp_b.tile([B, 2, HD], F32, name="pb")
            for c in range(2):
                nc.tensor.matmul(
                    pb[:, c, :], mixTb[:], bsc[:, c * HD:(c + 1) * HD],
                    start=True, stop=True,
                )
            bias_sb = _ctile(const, "bias_sb", [B, D], F32)
            for c in range(2):
                nc.scalar.copy(bias_sb[:, c * HD:(c + 1) * HD], pb[:, c, :])

            # ---- experts: acc_e = (x @ W_e) * mix_loc[:, e] + acc_{e-1} ----
            prev = bias_sb
            for e in range(EPC):
                pe = pp_e.tile([B, 2, HD], F32, name="pe")
                for k in range(KT):
                    for c in range(2):
                        nc.tensor.matmul(
                            pe[:, c, :], xtb[:, k, :],
                            wts[e][:, k, c * HD:(c + 1) * HD],
                            start=(k == 0), stop=(k == KT - 1),
                        )
                acc = _ctile(const, f"acc{e}", [B, D], F32)
                for c in range(2):
                    nc.vector.scalar_tensor_tensor(
                        acc[:, c * HD:(c + 1) * HD], pe[:, c, :], mix_loc[:, e:e + 1],
                        prev[:, c * HD:(c + 1) * HD], op0=ALU.mult, op1=ALU.add,
                    )
                prev = acc

            nc.sync.dma_start(out_d[:], prev[:])

    nc.finalize()
    return nc


def make_input_maps(x, router_w, bias_router_w, expert_weights, expert_biases):
    xt = np.ascontiguousarray(
        x.T.reshape(KT, 128, B).transpose(1, 0, 2), dtype=np.float32
    )
    xtb = np.ascontiguousarray(xt.astype(ml_dtypes.bfloat16))
    rw2 = np.ascontiguousarray(
        np.concatenate([router_w, bias_router_w], axis=1)
        .reshape(KT, 128, 2 * E)
        .transpose(1, 0, 2),
        dtype=np.float32,
    )
    bsc = (expert_biases / NCORES).astype(np.float32)
    ident = np.eye(128, dtype=np.float32)

    in_maps = []
    for c in range(NCORES):
        wl = (
            expert_weights[c * EPC:(c + 1) * EPC]
            .reshape(EPC, KT, 128, D)
            .transpose(0, 2, 1, 3)
        )
        wl = np.ascontiguousarray(wl).astype(ml_dtypes.bfloat16)
        selc = np.zeros((E, EPC), dtype=np.float32)
        for j in range(EPC):
            selc[c * EPC + j, j] = 1.0
        in_maps.append(
            dict(xt32=xt, xtb=xtb, rw2=rw2, wloc=wl, bsc=bsc, sel=selc, ident=ident)
        )
    return in_maps


def kernel(x, router_w, bias_router_w, expert_weights, expert_biases, **bench_kwargs):
    in_maps = make_input_maps(x, router_w, bias_router_w, expert_weights, expert_biases)
    nc = build_program()
    res = run_bass_kernel_spmd(nc, in_maps, list(range(NCORES)), **bench_kwargs)
    out = np.zeros((B, D), dtype=np.float64)
    for r in res.results:
        out += r["out"].astype(np.float64)
    final = out.astype(np.float32)
    if bench_kwargs:
        kernel.last_result = res
    return final
